# revision 6
# baseline (speedup 1.0000x reference)
"""Multi-head self-attention Trainium2 kernel (B=4, T=2048, D=512, H=8, HD=64).

Sharding: 8 cores = 4 batches x 2 head-groups (4 heads each). Each core:
  - transposes x[b] on the PE (fp32 exact),
  - projects q,k (transposed layout [c, t]) and v (natural [t, e], with 64
    ones-columns appended per head so the softmax denominator comes out of the
    PV matmul pre-broadcast across 64 partitions),
  - causal flash attention in "transposed space": S^T[j,i] blocks via
    row-tiled concurrent head-pair matmuls (K=64 fp32r), causal mask added
    inside PSUM via one extra matmul (U' lower-triangle const x identity),
    exp on ACT (scale=1/8 folded in, no max subtraction needed since
    |logits| <~ 8), PV accumulation in PSUM,
  - normalizes with reciprocal_approx_fast + tensor_mul,
  - output projection for its 256 contraction rows -> partial [2048, 512].
Host sums the 2 partials per batch (the TP all-reduce of the hint).

All matmuls run as float32r (TF32-class, ~1.3e-3 mean rel err, full PE rate).
"""

import sys

sys.path.insert(0, "/opt/trn_rl_repo")

import numpy as np

import concourse.bass as bass
import concourse.tile as tile
from concourse import bacc, masks, mybir
from concourse.bass_utils import run_bass_kernel_spmd

f32 = mybir.dt.float32
f32r = mybir.dt.float32r
u32 = mybir.dt.uint32

B, T, D, H, HD = 4, 2048, 512, 8, 64
NCORES = 8
SCALE = 1.0 / np.sqrt(HD)  # 0.125
NEG = -1.0e30

_BUILT = None  # cached (nc,)
DEBUG = False


def _build():
    nc = bacc.Bacc("TRN2", target_bir_lowering=False, debug=False)

    x_d = nc.dram_tensor("x", [T, D], f32, kind="ExternalInput")
    wqk_d = nc.dram_tensor("wqk", [D, 512], f32, kind="ExternalInput")
    wv_d = nc.dram_tensor("wv", [D, 256], f32, kind="ExternalInput")
    wo_d = nc.dram_tensor("wo", [256, D], f32, kind="ExternalInput")
    out_d = nc.dram_tensor("out", [T, D], f32, kind="ExternalOutput")
    dbg = {}
    if DEBUG:
        dbg["xT"] = nc.dram_tensor("dbg_xT", [128, 4, T], f32, kind="ExternalOutput")
        dbg["qkT"] = nc.dram_tensor("dbg_qkT", [128, 4, T], f32, kind="ExternalOutput")
        dbg["vaug"] = nc.dram_tensor("dbg_vaug", [128, 16, 4, 128], f32, kind="ExternalOutput")
        dbg["yTn"] = nc.dram_tensor("dbg_yTn", [128, 2, T], f32, kind="ExternalOutput")
        dbg["P"] = nc.dram_tensor("dbg_P", [128, 512], f32, kind="ExternalOutput")
        dbg["S"] = nc.dram_tensor("dbg_S", [128, 512], f32, kind="ExternalOutput")
        dbg["pv"] = nc.dram_tensor("dbg_pv", [128, 512], f32, kind="ExternalOutput")

    with tile.TileContext(nc) as tc:
        with (
            tc.tile_pool(name="big", bufs=1) as big,
            tc.tile_pool(name="xin", bufs=4) as xinp,
            tc.tile_pool(name="pp", bufs=6) as ppool,
            tc.tile_pool(name="rp", bufs=3) as rpool,
            tc.tile_pool(name="op", bufs=3) as opool,
            tc.tile_pool(name="ps", bufs=5, space=bass.MemorySpace.PSUM) as psA,
            tc.tile_pool(name="pv", bufs=3, space=bass.MemorySpace.PSUM) as psPV,
        ):
            # ---- persistent sbuf tensors ----
            xT = big.tile([128, 4, T], f32r)        # [d_part, d_chunk, t]
            qkT = big.tile([128, 4, T], f32r)       # ct: 0=q(h0,h1) 1=q(h2,h3) 2=k(h0,h1) 3=k(h2,h3)
            vaug = big.tile([128, 16, 4, 128], f32r)  # [t_part, t_tile, head, 64 v + 64 ones]
            yTn = big.tile([128, 2, T], f32r)       # [c_part, c_chunk, t]
            wqk_s = big.tile([128, 4, 512], f32r)
            wv_s = big.tile([128, 4, 256], f32r)
            wo_s = big.tile([128, 2, 512], f32r)

            # ---- constants ----
            ident = big.tile([128, 128], f32)
            masks.make_identity(nc, ident[:])
            ones256 = big.tile([128, 256], f32)
            nc.vector.memset(ones256[:], 1.0)
            up_f = big.tile([128, 128], f32)
            masks.make_causal_mask(nc, up_f[:], mask_val=NEG)  # [k,m]=NEG where m>k
            uprime = big.tile([128, 128], f32r)
            nc.vector.tensor_scalar_mul(uprime[:], up_f[:], 1.0)
            eid_f = big.tile([128, 512], f32)
            nc.vector.memset(eid_f[:], 0.0)
            masks.make_identity(nc, eid_f[:, 0:128], nomemset=True)
            eident = big.tile([128, 512], f32r)
            nc.vector.tensor_scalar_mul(eident[:], eid_f[:], 1.0)
            zero_f = big.tile([128, 384], f32)
            nc.vector.memset(zero_f[:], 0.0)
            zero_r = big.tile([128, 384], f32r)
            nc.vector.tensor_scalar_mul(zero_r[:], zero_f[:], 1.0)

            # ---- weight loads (bitcast: fp32r consumes raw fp32 bits) ----
            nc.sync.dma_start(
                wqk_s[:], wqk_d.ap().rearrange("(c p) m -> p c m", p=128).bitcast(f32r)
            )
            nc.sync.dma_start(
                wv_s[:], wv_d.ap().rearrange("(c p) m -> p c m", p=128).bitcast(f32r)
            )
            nc.sync.dma_start(
                wo_s[:], wo_d.ap().rearrange("(c p) m -> p c m", p=128).bitcast(f32r)
            )

            # ---- phase 1: load x tiles + transpose into xT ----
            for tt in range(16):
                xt = xinp.tile([128, 512], f32)
                nc.sync.dma_start(xt[:], x_d.ap()[tt * 128:(tt + 1) * 128, :])
                for c in range(4):
                    ptr = psA.tile([128, 128], f32, tag="A")
                    nc.tensor.transpose(ptr[:], xt[:, c * 128:(c + 1) * 128], ident[:])
                    nc.vector.tensor_scalar_mul(
                        xT[:, c, tt * 128:(tt + 1) * 128], ptr[:], 1.0
                    )

            # ---- phase 2a: q,k projection (transposed layout) ----
            for ct in range(4):
                for t4 in range(4):
                    ps = psA.tile([128, 512], f32, tag="A")
                    for c in range(4):
                        nc.tensor.matmul(
                            ps[:],
                            wqk_s[:, c, ct * 128:(ct + 1) * 128],
                            xT[:, c, t4 * 512:(t4 + 1) * 512],
                            start=(c == 0),
                            stop=(c == 3),
                        )
                    nc.vector.tensor_scalar_mul(
                        qkT[:, ct, t4 * 512:(t4 + 1) * 512], ps[:], 1.0
                    )

            # ---- phase 2b: v projection (natural layout) + ones columns ----
            for tt in range(16):
                psv = psA.tile([128, 256], f32, tag="A")
                for c in range(4):
                    nc.tensor.matmul(
                        psv[:],
                        xT[:, c, tt * 128:(tt + 1) * 128],
                        wv_s[:, c, :],
                        start=(c == 0),
                        stop=(c == 3),
                    )
                nc.vector.tensor_scalar_mul(
                    vaug[:, tt, :, 0:64],
                    psv[:].rearrange("p (h e) -> p h e", e=64),
                    1.0,
                )
                nc.vector.tensor_scalar_mul(
                    vaug[:, tt, :, 64:128],
                    ones256[:].rearrange("p (h e) -> p h e", e=64),
                    1.0,
                )

            # ---- phase 3: causal attention, head pairs ----
            for p in range(2):  # heads (2p, 2p+1)
                for it in range(4):
                    i0 = it * 512
                    pvA = psPV.tile([128, 512], f32, tag="PV")
                    pvB = psPV.tile([128, 512], f32, tag="PV")
                    njc = 4 * it + 4
                    for jc in range(njc):
                        s = jc - 4 * it  # >=0 on diagonal chunks
                        diag = s >= 0
                        W = 512 - 128 * s if diag else 512
                        kA = qkT[0:64, 2 + p, jc * 128:(jc + 1) * 128]
                        kB = qkT[64:128, 2 + p, jc * 128:(jc + 1) * 128]
                        qA = qkT[0:64, p, i0 + (512 - W):i0 + 512]
                        qB = qkT[64:128, p, i0 + (512 - W):i0 + 512]
                        sA = psA.tile([128, W], f32, tag="A")
                        sB = psA.tile([128, W], f32, tag="A")
                        nc.tensor.matmul(
                            sA[:], kA, qA, start=True, stop=not diag,
                            tile_position=(0, 0),
                        )
                        nc.tensor.matmul(
                            sB[:], kB, qB, start=True, stop=not diag,
                            tile_position=(64, 0),
                        )
                        if diag:
                            nc.tensor.matmul(
                                sA[:], uprime[:], eident[:, 0:W],
                                start=False, stop=True,
                            )
                            nc.tensor.matmul(
                                sB[:], uprime[:], eident[:, 0:W],
                                start=False, stop=True,
                            )
                        if DEBUG and p == 0 and it == 0 and jc == 0:
                            s_dump = opool.tile([128, 512], f32, tag="o")
                            nc.vector.tensor_scalar_mul(s_dump[:, 0:W], sA[:], 1.0)
                            nc.sync.dma_start(dbg["S"].ap()[:, 0:W], s_dump[:, 0:W])
                        pA = ppool.tile([128, 512], f32r, tag="P")
                        pB = ppool.tile([128, 512], f32r, tag="P")
                        if diag and s > 0:
                            nc.gpsimd.memset(pA[:, 0:512 - W].bitcast(u32), 0)
                            nc.gpsimd.memset(pB[:, 0:512 - W].bitcast(u32), 0)
                        nc.scalar.activation(
                            pA[:, 512 - W:512], sA[:],
                            mybir.ActivationFunctionType.Exp, scale=SCALE,
                        )
                        nc.scalar.activation(
                            pB[:, 512 - W:512], sB[:],
                            mybir.ActivationFunctionType.Exp, scale=SCALE,
                        )
                        if DEBUG and p == 0 and it == 0 and jc == 0:
                            nc.sync.dma_start(dbg["P"].ap().bitcast(f32r), pA[:])
                        nc.tensor.matmul(
                            pvA[:], vaug[:, jc, 2 * p, :], pA[:],
                            start=(jc == 0), stop=(jc == njc - 1),
                        )
                        nc.tensor.matmul(
                            pvB[:], vaug[:, jc, 2 * p + 1, :], pB[:],
                            start=(jc == 0), stop=(jc == njc - 1),
                        )
                    if DEBUG and p == 0 and it == 0:
                        pv_dump = opool.tile([128, 512], f32, tag="o")
                        nc.vector.tensor_scalar_mul(pv_dump[:], pvA[:], 1.0)
                        nc.sync.dma_start(dbg["pv"].ap()[:], pv_dump[:])
                    # normalize: rows 64:128 hold l replicated 64x
                    lsA = rpool.tile([64, 512], f32, tag="l")
                    nc.vector.tensor_scalar_mul(lsA[:], pvA[64:128, :], 1.0)
                    rA = rpool.tile([64, 512], f32, tag="r")
                    nc.vector.reciprocal_approx_fast(rA[:], lsA[:])
                    nc.vector.tensor_mul(
                        yTn[0:64, p, i0:i0 + 512], pvA[0:64, :], rA[:]
                    )
                    lsB = rpool.tile([64, 512], f32, tag="l")
                    nc.vector.tensor_scalar_mul(lsB[:], pvB[64:128, :], 1.0)
                    rB = rpool.tile([64, 512], f32, tag="r")
                    nc.vector.reciprocal_approx_fast(rB[:], lsB[:])
                    nc.vector.tensor_mul(
                        yTn[64:128, p, i0:i0 + 512], pvB[0:64, :], rB[:]
                    )

            # ---- phase 4: output projection ----
            for tt in range(16):
                po = psA.tile([128, 512], f32, tag="A")
                nc.tensor.matmul(
                    po[:], yTn[:, 0, tt * 128:(tt + 1) * 128], wo_s[:, 0, :],
                    start=True, stop=False,
                )
                nc.tensor.matmul(
                    po[:], yTn[:, 1, tt * 128:(tt + 1) * 128], wo_s[:, 1, :],
                    start=False, stop=True,
                )
                ot = opool.tile([128, 512], f32, tag="o")
                nc.scalar.copy(ot[:], po[:])
                nc.sync.dma_start(out_d.ap()[tt * 128:(tt + 1) * 128, :], ot[:])

            if DEBUG:
                nc.sync.dma_start(dbg["xT"].ap().bitcast(f32r), xT[:])
                nc.sync.dma_start(dbg["qkT"].ap().bitcast(f32r), qkT[:])
                nc.sync.dma_start(dbg["vaug"].ap().bitcast(f32r), vaug[:])
                nc.sync.dma_start(dbg["yTn"].ap().bitcast(f32r), yTn[:])

    nc.compile()
    return nc


def _get_nc():
    global _BUILT
    if _BUILT is None:
        _BUILT = _build()
    return _BUILT


def _make_in_maps(x, Wqkv, Wout):
    q, k, v = Wqkv[:, 0:512], Wqkv[:, 512:1024], Wqkv[:, 1024:1536]
    in_maps = []
    for core in range(NCORES):
        b, g = core // 2, core % 2
        hs = [g * 4 + i for i in range(4)]
        wqk = np.concatenate(
            [q[:, h * 64:(h + 1) * 64] for h in hs]
            + [k[:, h * 64:(h + 1) * 64] for h in hs],
            axis=1,
        )
        wv = np.ascontiguousarray(v[:, g * 256:(g + 1) * 256])
        wo = np.ascontiguousarray(Wout[g * 256:(g + 1) * 256, :])
        in_maps.append(
            {
                "x": np.ascontiguousarray(x[b]),
                "wqk": np.ascontiguousarray(wqk),
                "wv": wv,
                "wo": wo,
            }
        )
    return in_maps


def _run(x, Wqkv, Wout, trace=False):
    nc = _get_nc()
    in_maps = _make_in_maps(x, Wqkv, Wout)
    res = run_bass_kernel_spmd(
        nc, in_maps, core_ids=list(range(NCORES)), trace=trace
    )
    out = np.empty((B, T, D), dtype=np.float32)
    for b in range(B):
        out[b] = res.results[2 * b]["out"] + res.results[2 * b + 1]["out"]
    return out, res


def _reference_fallback(x, attn_mask, Wqkv, Wout):
    # general (non-causal-mask) path: plain numpy
    qkv = x @ Wqkv
    q, k, v = np.split(qkv, 3, axis=-1)

    def heads(t):
        return t.reshape(B, T, H, HD).transpose(0, 2, 1, 3)

    q, k, v = heads(q), heads(k), heads(v)
    att = np.einsum("bhqd,bhkd->bhqk", q, k) * SCALE
    att = np.where(attn_mask[None, None] == 0, -np.inf, att)
    att = att - att.max(axis=-1, keepdims=True)
    att = np.exp(att)
    att = att / att.sum(axis=-1, keepdims=True)
    y = np.einsum("bhqk,bhkd->bhqd", att, v)
    return (y.transpose(0, 2, 1, 3).reshape(B, T, D) @ Wout).astype(np.float32)


def kernel(x, attn_mask, Wqkv, Wout):
    x = np.asarray(x, dtype=np.float32)
    attn_mask = np.asarray(attn_mask)
    Wqkv = np.asarray(Wqkv, dtype=np.float32)
    Wout = np.asarray(Wout, dtype=np.float32)

    causal = bool(
        np.array_equal(attn_mask != 0, np.tril(np.ones((T, T), dtype=bool)))
    )
    if not causal:
        return _reference_fallback(x, attn_mask, Wqkv, Wout)

    out, _ = _run(x, Wqkv, Wout, trace=False)
    return out


# revision 8
# speedup vs baseline: 1.1886x; 1.1886x over previous
"""Multi-head self-attention Trainium2 kernel (B=4, T=2048, D=512, H=8, HD=64).

Sharding: 8 cores = 4 batches x 2 head-groups (4 heads each). Each core:
  - transposes x[b] on the PE (fp32 exact),
  - projects q,k (transposed layout [c, t]) and v (natural [t, e], with 64
    ones-columns appended per head so the softmax denominator comes out of the
    PV matmul pre-broadcast across 64 partitions),
  - causal flash attention in "transposed space": S^T[j,i] blocks via
    row-tiled concurrent head-pair matmuls (K=64 fp32r), causal mask added
    inside PSUM via one extra matmul (U' lower-triangle const x identity),
    exp on ACT (scale=1/8 folded in, no max subtraction needed since
    |logits| <~ 8), PV accumulation in PSUM,
  - normalizes with reciprocal_approx_fast + tensor_mul,
  - output projection for its 256 contraction rows -> partial [2048, 512].
Host sums the 2 partials per batch (the TP all-reduce of the hint).

All matmuls run as float32r (TF32-class, ~1.3e-3 mean rel err, full PE rate).
"""

import sys

sys.path.insert(0, "/opt/trn_rl_repo")

import numpy as np

import concourse.bass as bass
import concourse.tile as tile
from concourse import bacc, masks, mybir
from concourse.bass_utils import run_bass_kernel_spmd

f32 = mybir.dt.float32
f32r = mybir.dt.float32r
u32 = mybir.dt.uint32

B, T, D, H, HD = 4, 2048, 512, 8, 64
NCORES = 8
SCALE = 1.0 / np.sqrt(HD)  # 0.125
NEG = -1.0e30

_BUILT = None  # cached (nc,)
DEBUG = False


def _build():
    nc = bacc.Bacc("TRN2", target_bir_lowering=False, debug=False)

    x_d = nc.dram_tensor("x", [T, D], f32, kind="ExternalInput")
    wqk_d = nc.dram_tensor("wqk", [D, 512], f32, kind="ExternalInput")
    wv_d = nc.dram_tensor("wv", [D, 256], f32, kind="ExternalInput")
    wo_d = nc.dram_tensor("wo", [256, D], f32, kind="ExternalInput")
    out_d = nc.dram_tensor("out", [T, D], f32, kind="ExternalOutput")
    dbg = {}
    if DEBUG:
        dbg["xT"] = nc.dram_tensor("dbg_xT", [128, 4, T], f32, kind="ExternalOutput")
        dbg["qkT"] = nc.dram_tensor("dbg_qkT", [128, 4, T], f32, kind="ExternalOutput")
        dbg["vaug"] = nc.dram_tensor("dbg_vaug", [128, 16, 4, 128], f32, kind="ExternalOutput")
        dbg["yTn"] = nc.dram_tensor("dbg_yTn", [128, 2, T], f32, kind="ExternalOutput")

    with tile.TileContext(nc) as tc:
        with (
            tc.tile_pool(name="big", bufs=1) as big,
            tc.tile_pool(name="xin", bufs=4) as xinp,
            tc.tile_pool(name="pp", bufs=6) as ppool,
            tc.tile_pool(name="rp", bufs=3) as rpool,
            tc.tile_pool(name="op", bufs=3) as opool,
            tc.tile_pool(name="ps", bufs=2, space=bass.MemorySpace.PSUM) as psA,
            tc.tile_pool(name="ps2", bufs=2, space=bass.MemorySpace.PSUM) as psS2,
            tc.tile_pool(name="pv", bufs=2, space=bass.MemorySpace.PSUM) as psPV,
        ):
            # ---- persistent sbuf tensors ----
            xT = big.tile([128, 4, T], f32r)        # [d_part, d_chunk, t]
            qkT = big.tile([128, 4, T], f32r)       # ct: 0=q(h0,h1) 1=q(h2,h3) 2=k(h0,h1) 3=k(h2,h3)
            vaug = big.tile([128, 16, 4, 128], f32r)  # [t_part, t_tile, head, 64 v + 64 ones]
            yTn = big.tile([128, 2, T], f32r)       # [c_part, c_chunk, t]
            wqk_s = big.tile([128, 4, 512], f32r)
            wv_s = big.tile([128, 4, 256], f32r)
            wo_s = big.tile([128, 2, 512], f32r)

            # ---- constants ----
            ident = big.tile([128, 128], f32)
            masks.make_identity(nc, ident[:])
            ones256 = big.tile([128, 256], f32)
            nc.vector.memset(ones256[:], 1.0)
            up_f = big.tile([128, 128], f32)
            masks.make_causal_mask(nc, up_f[:], mask_val=NEG)  # [k,m]=NEG where m>k
            uprime = big.tile([128, 128], f32r)
            nc.vector.tensor_scalar_mul(uprime[:], up_f[:], 1.0)
            eid_f = big.tile([128, 512], f32)
            nc.vector.memset(eid_f[:], 0.0)
            masks.make_identity(nc, eid_f[:, 0:128], nomemset=True)
            eident = big.tile([128, 512], f32r)
            nc.vector.tensor_scalar_mul(eident[:], eid_f[:], 1.0)
            zero_f = big.tile([128, 384], f32)
            nc.vector.memset(zero_f[:], 0.0)
            zero_r = big.tile([128, 384], f32r)
            nc.vector.tensor_scalar_mul(zero_r[:], zero_f[:], 1.0)

            # ---- weight loads (bitcast: fp32r consumes raw fp32 bits) ----
            nc.sync.dma_start(
                wqk_s[:], wqk_d.ap().rearrange("(c p) m -> p c m", p=128).bitcast(f32r)
            )
            nc.sync.dma_start(
                wv_s[:], wv_d.ap().rearrange("(c p) m -> p c m", p=128).bitcast(f32r)
            )
            nc.sync.dma_start(
                wo_s[:], wo_d.ap().rearrange("(c p) m -> p c m", p=128).bitcast(f32r)
            )

            # ---- phase 1: load x tiles + transpose into xT ----
            for tt in range(16):
                xt = xinp.tile([128, 512], f32)
                nc.sync.dma_start(xt[:], x_d.ap()[tt * 128:(tt + 1) * 128, :])
                for c in range(4):
                    ptr = psA.tile([128, 128], f32, tag="A")
                    nc.tensor.transpose(ptr[:], xt[:, c * 128:(c + 1) * 128], ident[:])
                    nc.vector.tensor_scalar_mul(
                        xT[:, c, tt * 128:(tt + 1) * 128], ptr[:], 1.0
                    )

            # ---- phase 2a: q,k projection (transposed layout) ----
            for ct in range(4):
                for t4 in range(4):
                    ps = psA.tile([128, 512], f32, tag="A")
                    for c in range(4):
                        nc.tensor.matmul(
                            ps[:],
                            wqk_s[:, c, ct * 128:(ct + 1) * 128],
                            xT[:, c, t4 * 512:(t4 + 1) * 512],
                            start=(c == 0),
                            stop=(c == 3),
                        )
                    nc.vector.tensor_scalar_mul(
                        qkT[:, ct, t4 * 512:(t4 + 1) * 512], ps[:], 1.0
                    )

            # ---- phase 2b: v projection (natural layout) + ones columns ----
            for tt in range(16):
                psv = psA.tile([128, 256], f32, tag="A")
                for c in range(4):
                    nc.tensor.matmul(
                        psv[:],
                        xT[:, c, tt * 128:(tt + 1) * 128],
                        wv_s[:, c, :],
                        start=(c == 0),
                        stop=(c == 3),
                    )
                nc.vector.tensor_scalar_mul(
                    vaug[:, tt, :, 0:64],
                    psv[:].rearrange("p (h e) -> p h e", e=64),
                    1.0,
                )
                nc.vector.tensor_scalar_mul(
                    vaug[:, tt, :, 64:128],
                    ones256[:].rearrange("p (h e) -> p h e", e=64),
                    1.0,
                )

            # ---- phase 3: causal attention, head pairs ----
            for p in range(2):  # heads (2p, 2p+1)
                for it in range(4):
                    i0 = it * 512
                    pvA = psPV.tile([128, 512], f32, tag="PV")
                    pvB = psPV.tile([128, 512], f32, tag="PV")
                    njc = 4 * it + 4
                    for jc in range(njc):
                        s = jc - 4 * it  # >=0 on diagonal chunks
                        diag = s >= 0
                        W = 512 - 128 * s if diag else 512
                        kA = qkT[0:64, 2 + p, jc * 128:(jc + 1) * 128]
                        kB = qkT[64:128, 2 + p, jc * 128:(jc + 1) * 128]
                        qA = qkT[0:64, p, i0 + (512 - W):i0 + 512]
                        qB = qkT[64:128, p, i0 + (512 - W):i0 + 512]
                        if not diag:
                            sAB = psS2.tile([128, 1024], f32, tag="S2")
                            nc.tensor.matmul(
                                sAB[:, 0:512], kA, qA, start=True, stop=True,
                                tile_position=(0, 0),
                            )
                            nc.tensor.matmul(
                                sAB[:, 512:1024], kB, qB, start=True, stop=True,
                                tile_position=(64, 0),
                            )
                            pAB = ppool.tile([128, 1024], f32r, tag="P2")
                            nc.scalar.activation(
                                pAB[:], sAB[:],
                                mybir.ActivationFunctionType.Exp, scale=SCALE,
                            )
                            rhsA, rhsB = pAB[:, 0:512], pAB[:, 512:1024]
                        else:
                            sA = psA.tile([128, W], f32, tag="A")
                            sB = psA.tile([128, W], f32, tag="A")
                            nc.tensor.matmul(
                                sA[:], kA, qA, start=True, stop=False,
                                tile_position=(0, 0),
                            )
                            nc.tensor.matmul(
                                sB[:], kB, qB, start=True, stop=False,
                                tile_position=(64, 0),
                            )
                            nc.tensor.matmul(
                                sA[:], uprime[:], eident[:, 0:W],
                                start=False, stop=True,
                            )
                            nc.tensor.matmul(
                                sB[:], uprime[:], eident[:, 0:W],
                                start=False, stop=True,
                            )
                            pA = ppool.tile([128, 512], f32r, tag="P")
                            pB = ppool.tile([128, 512], f32r, tag="P")
                            if s > 0:
                                nc.gpsimd.memset(pA[:, 0:512 - W].bitcast(u32), 0)
                                nc.gpsimd.memset(pB[:, 0:512 - W].bitcast(u32), 0)
                            nc.scalar.activation(
                                pA[:, 512 - W:512], sA[:],
                                mybir.ActivationFunctionType.Exp, scale=SCALE,
                            )
                            nc.scalar.activation(
                                pB[:, 512 - W:512], sB[:],
                                mybir.ActivationFunctionType.Exp, scale=SCALE,
                            )
                            rhsA, rhsB = pA[:], pB[:]
                        nc.tensor.matmul(
                            pvA[:], vaug[:, jc, 2 * p, :], rhsA,
                            start=(jc == 0), stop=(jc == njc - 1),
                        )
                        nc.tensor.matmul(
                            pvB[:], vaug[:, jc, 2 * p + 1, :], rhsB,
                            start=(jc == 0), stop=(jc == njc - 1),
                        )
                    # normalize: rows 64:128 hold l replicated 64x
                    lsA = rpool.tile([64, 512], f32, tag="l")
                    nc.vector.tensor_scalar_mul(lsA[:], pvA[64:128, :], 1.0)
                    rA = rpool.tile([64, 512], f32, tag="r")
                    nc.vector.reciprocal_approx_fast(rA[:], lsA[:])
                    nc.vector.tensor_mul(
                        yTn[0:64, p, i0:i0 + 512], pvA[0:64, :], rA[:]
                    )
                    lsB = rpool.tile([64, 512], f32, tag="l")
                    nc.vector.tensor_scalar_mul(lsB[:], pvB[64:128, :], 1.0)
                    rB = rpool.tile([64, 512], f32, tag="r")
                    nc.vector.reciprocal_approx_fast(rB[:], lsB[:])
                    nc.vector.tensor_mul(
                        yTn[64:128, p, i0:i0 + 512], pvB[0:64, :], rB[:]
                    )

            # ---- phase 4: output projection ----
            for tt in range(16):
                po = psA.tile([128, 512], f32, tag="A")
                nc.tensor.matmul(
                    po[:], yTn[:, 0, tt * 128:(tt + 1) * 128], wo_s[:, 0, :],
                    start=True, stop=False,
                )
                nc.tensor.matmul(
                    po[:], yTn[:, 1, tt * 128:(tt + 1) * 128], wo_s[:, 1, :],
                    start=False, stop=True,
                )
                ot = opool.tile([128, 512], f32, tag="o")
                nc.vector.tensor_scalar_mul(ot[:], po[:], 1.0)
                nc.sync.dma_start(out_d.ap()[tt * 128:(tt + 1) * 128, :], ot[:])

            if DEBUG:
                nc.sync.dma_start(dbg["xT"].ap().bitcast(f32r), xT[:])
                nc.sync.dma_start(dbg["qkT"].ap().bitcast(f32r), qkT[:])
                nc.sync.dma_start(dbg["vaug"].ap().bitcast(f32r), vaug[:])
                nc.sync.dma_start(dbg["yTn"].ap().bitcast(f32r), yTn[:])

    nc.compile()
    return nc


def _get_nc():
    global _BUILT
    if _BUILT is None:
        _BUILT = _build()
    return _BUILT


def _make_in_maps(x, Wqkv, Wout):
    q, k, v = Wqkv[:, 0:512], Wqkv[:, 512:1024], Wqkv[:, 1024:1536]
    in_maps = []
    for core in range(NCORES):
        b, g = core // 2, core % 2
        hs = [g * 4 + i for i in range(4)]
        wqk = np.concatenate(
            [q[:, h * 64:(h + 1) * 64] for h in hs]
            + [k[:, h * 64:(h + 1) * 64] for h in hs],
            axis=1,
        )
        wv = np.ascontiguousarray(v[:, g * 256:(g + 1) * 256])
        wo = np.ascontiguousarray(Wout[g * 256:(g + 1) * 256, :])
        in_maps.append(
            {
                "x": np.ascontiguousarray(x[b]),
                "wqk": np.ascontiguousarray(wqk),
                "wv": wv,
                "wo": wo,
            }
        )
    return in_maps


def _run(x, Wqkv, Wout, trace=False):
    nc = _get_nc()
    in_maps = _make_in_maps(x, Wqkv, Wout)
    res = run_bass_kernel_spmd(
        nc, in_maps, core_ids=list(range(NCORES)), trace=trace
    )
    out = np.empty((B, T, D), dtype=np.float32)
    for b in range(B):
        out[b] = res.results[2 * b]["out"] + res.results[2 * b + 1]["out"]
    return out, res


def _reference_fallback(x, attn_mask, Wqkv, Wout):
    # general (non-causal-mask) path: plain numpy
    qkv = x @ Wqkv
    q, k, v = np.split(qkv, 3, axis=-1)

    def heads(t):
        return t.reshape(B, T, H, HD).transpose(0, 2, 1, 3)

    q, k, v = heads(q), heads(k), heads(v)
    att = np.einsum("bhqd,bhkd->bhqk", q, k) * SCALE
    att = np.where(attn_mask[None, None] == 0, -np.inf, att)
    att = att - att.max(axis=-1, keepdims=True)
    att = np.exp(att)
    att = att / att.sum(axis=-1, keepdims=True)
    y = np.einsum("bhqk,bhkd->bhqd", att, v)
    return (y.transpose(0, 2, 1, 3).reshape(B, T, D) @ Wout).astype(np.float32)


def kernel(x, attn_mask, Wqkv, Wout):
    x = np.asarray(x, dtype=np.float32)
    attn_mask = np.asarray(attn_mask)
    Wqkv = np.asarray(Wqkv, dtype=np.float32)
    Wout = np.asarray(Wout, dtype=np.float32)

    causal = bool(
        np.array_equal(attn_mask != 0, np.tril(np.ones((T, T), dtype=bool)))
    )
    if not causal:
        return _reference_fallback(x, attn_mask, Wqkv, Wout)

    out, _ = _run(x, Wqkv, Wout, trace=False)
    return out
